# revision 34
# baseline (speedup 1.0000x reference)
"""HCNN (known-U) recurrence kernel for 8 Trainium2 NeuronCores.

Model (see reference): 80 sequential steps of
    state' = tanh(cat(post_state, u)) @ A            A: (2112, 2048) fp32
with teacher forcing post_state[:, :128] = y during the 64 past steps,
outputs = 64 past errors then 16 forecasts (first 128 state components).

Strategy
--------
Data-parallel over batch: 256 = 8 cores x 32. Each core runs the full
recurrence for its batch slice; no collectives.

Per-core per-step matmul x @ A with batch M=32 would waste 3/4 of the
128-wide PE array, so the A columns are split into 4 interleaved groups
and computed by 4 concurrent column-tiled matmuls (tile_position=(0,32j))
sharing the array. Data is fp16 (single pass): the teacher-forced
recurrence is strongly contractive; end-to-end output error ~1.5e-4
relative vs the fp32 reference.

Column interleave: state column s lives in col-group j=(s//32)%4 at free
offset 32*(s//128) + s%32. The (128, 512) psum holding state' (batch on
partitions within each 32-group) turns into the next step's stationary
operand layout via DVE 32x32 block-transposes: block (j, m') lands at
partitions [32j:32j+32] of k-tile m' -- exactly where matmul round m'
reads it.

Pipelining (the point of this version): each step's 17 k-rounds are
split into lo (psum cols 0:256 = k-tiles 0..7 of the next x) and hi
(cols 256:512 = k-tiles 8..15) accumulation groups. The lo group stops
half way through the step, so ACT tanh + DVE transpose of the lo chunks
overlap the hi matmuls, and the hi chunks' tanh/transpose overlaps the
next step's early rounds (which consume y/u/lo-chunk operands first).
The PE never waits on the full tanh->transpose chain. A tiles are
DMA-streamed in first-use order so step 0 starts as soon as tile k=0
lands; outputs are DMA'd out in slices as they are produced.
"""

import sys

for _p in ("/opt/trn_rl_repo", "/root/.axon_site/_ro/trn_rl_repo"):
    if _p not in sys.path:
        sys.path.insert(0, _p)

import numpy as np

N_STATE = 2048
N_U = 64
N_Y = 128
PAST = 64
FORE = 16
BATCH = 256
T = PAST + FORE          # 80 total steps; only 79 matmul steps needed
NSTEP = T - 1            # step t computes state_{t+1}; state_80 is unused
NK = 17                  # contraction tiles: 16 x 128 state + 1 x (64 u + 64 pad)
KDIM = NK * 128          # 2176 padded contraction size
N_CORES = 8
B = BATCH // N_CORES     # 32 per core
OUT_CHUNK = 10           # output DMA granularity in steps

# psum slot permutation: slot p of the 16 32-col psum slots holds state
# k-tile TILE_OF_SLOT[p]. Tiles 1..9 form the lo accumulation group (cols
# 0:288); tile 0 (the output/forecast tile) leads the hi group, 10..15 trail.
TILE_OF_SLOT = [1, 2, 3, 4, 5, 6, 7, 8, 9, 0, 10, 11, 12, 13, 14, 15]
SLOT_OF_TILE = [0] * 16
for _p, _m in enumerate(TILE_OF_SLOT):
    SLOT_OF_TILE[_m] = _p
LO = 288                 # lo accumulation group column count


def _build_program():
    import concourse.bass as bass
    import concourse.tile as tile
    from concourse import mybir

    F32 = mybir.dt.float32
    F16 = mybir.dt.float16

    nc = bass.Bass("TRN2", target_bir_lowering=False, debug=False,
                   num_devices=N_CORES)

    A_ext = nc.declare_dram_parameter("A_re", [KDIM, 4, 512], F16, isOutput=False)
    contrib_ext = nc.declare_dram_parameter("contrib", [128, NSTEP * 512], F16, isOutput=False)
    ywrap_ext = nc.declare_dram_parameter("ywrap", [128, (PAST - 1) * B], F32, isOutput=False)
    initxT_ext = nc.declare_dram_parameter("initxT", [128, 512], F16, isOutput=False)
    out_ext = nc.declare_dram_parameter("outbuf", [128, NSTEP * B], F32, isOutput=True)

    # The y/u contributions to every step are known ahead of time; the host
    # computes them exactly and they are DVE-preloaded into PSUM, so the
    # device rounds cover only the state tiles (start=False accumulation on
    # top of the preload). The lo group's tiles are produced and
    # tanh/transposed a half-phase early; the k emission order consumes them
    # first, giving the late hi chunks ~1.2us of cover for the
    # stop->tanh->transpose chain.
    korder_past = list(range(1, 16))
    korder_fore = list(range(1, 10)) + [0] + list(range(10, 16))
    CCH = 8 * 512            # contrib DMA chunk: 8 steps

    with tile.TileContext(nc) as tc:
        with tc.tile_pool(name="const", bufs=1) as cpool, \
             tc.tile_pool(name="xbuf", bufs=2) as xpool, \
             tc.tile_pool(name="th", bufs=2) as thpool, \
             tc.tile_pool(name="psum", bufs=2, space="PSUM") as pspool:

            xT = xpool.tile([128, 512], F16, tag="xT")
            nc.sync.dma_start(out=xT[:], in_=initxT_ext[:])
            contrib = cpool.tile([128, NSTEP * 512], F16, tag="cb")
            # step 0's slice alone first: the initial PSUM preloads (and with
            # them all of step 0) would otherwise wait on a megabyte chunk
            nc.sync.dma_start(out=contrib[:, 0:512], in_=contrib_ext[:, 0:512])
            nc.sync.dma_start(out=contrib[:, 512:CCH], in_=contrib_ext[:, 512:CCH])

            # A k-tiles as separate tiles, DMA'd in first-use order so each
            # round of step 0 waits only for its own tile. Tile 0 is only
            # used from the first forecast-input step, so it loads last.
            A_sb = [None] * NK
            for i, k in enumerate(korder_past + [0]):
                A_sb[k] = cpool.tile([128, 2048], F16, tag=f"A{k}", name=f"A{k}")
                nc.sync.dma_start(out=A_sb[k][:],
                                  in_=A_ext[128 * k:128 * (k + 1), :, :])
                if i == 8:
                    nc.sync.dma_start(out=contrib[:, CCH:2 * CCH],
                                      in_=contrib_ext[:, CCH:2 * CCH])

            ywrap = cpool.tile([128, (PAST - 1) * B], F32, tag="yw")
            nc.sync.dma_start(out=ywrap[:], in_=ywrap_ext[:])
            for c in range(2, (NSTEP * 512 + CCH - 1) // CCH):
                nc.sync.dma_start(
                    out=contrib[:, c * CCH:min((c + 1) * CCH, NSTEP * 512)],
                    in_=contrib_ext[:, c * CCH:min((c + 1) * CCH, NSTEP * 512)])
            outbuf = cpool.tile([128, NSTEP * B], F32, tag="ob")

            def lhs_for(k, x):
                p = SLOT_OF_TILE[k]
                return x[:, 32 * p:32 * (p + 1)]

            ps_lo = pspool.tile([128, 512], F32, tag="plo")
            ps_hi = pspool.tile([128, 512], F32, tag="phi")
            nc.vector.tensor_copy(ps_lo[:, 0:LO], contrib[:, 0:LO])
            nc.vector.tensor_copy(ps_hi[:, 0:512 - LO], contrib[:, LO:512])

            out_done = 0
            for t in range(NSTEP):
                korder = korder_past if t < PAST else korder_fore
                last = t == NSTEP - 1
                if not last:
                    nxT = xpool.tile([128, 512], F16, tag="xT", name="nxT")
                    nps_lo = pspool.tile([128, 512], F32, tag="plo", name="nps_lo")
                    nps_hi = pspool.tile([128, 512], F32, tag="phi", name="nps_hi")

                # the last step only needs output cols 0:128 (= psum slot 9 =
                # hi cols 0:32 of each quadrant); its lo half is never run.
                halves = ((0, ps_lo), (1, ps_hi)) if not last else ((1, ps_hi),)
                for half, ps in halves:
                    c0, w = (0, LO) if half == 0 else (LO, 512 - LO)
                    for idx, k in enumerate(korder):
                        lhsT = lhs_for(k, xT)
                        stop = idx == len(korder) - 1
                        for j in range(4):
                            nc.tensor.matmul(
                                ps[32 * j:32 * (j + 1), 0:w],
                                lhsT,
                                A_sb[k][:, 512 * j + c0:512 * j + c0 + w],
                                start=False, stop=stop,
                                tile_position=(0, 32 * j),
                            )
                    if last:
                        continue  # last state: only the output copy is needed
                    if half == 0 and t + 1 < NSTEP - 1:
                        # preload next step's lo psum while this step's hi
                        # phase runs (DVE, ahead of the transposes: those of
                        # the lo half have a full phase of slack)
                        nc.vector.tensor_copy(
                            nps_lo[:, 0:LO],
                            contrib[:, 512 * (t + 1):512 * (t + 1) + LO])
                    # tanh + 32x32 block transpose of this half's slots into
                    # the next step's stationary operand tile. The hi chunks
                    # are on the next step's critical path: keep them 64 cols
                    # for low latency. Slot 9 (tile 0) is only needed once
                    # teacher forcing ends (t >= PAST-1).
                    if half == 0:
                        chunks = ((0, 128), (128, 288))
                    elif t < PAST - 1:
                        chunks = ((320, 384), (384, 512))
                    else:
                        chunks = ((288, 352), (352, 512))
                    for a, b_ in chunks:
                        th = thpool.tile([128, b_ - a], F16, tag=f"th{half}{a}")
                        nc.scalar.activation(th[:], ps[:, a - c0:b_ - c0],
                                             mybir.ActivationFunctionType.Tanh)
                        nc.vector.transpose(nxT[:, a:b_], th[:])
                if not last:
                    nc.vector.tensor_copy(
                        nps_hi[:, 0:512 - LO],
                        contrib[:, 512 * (t + 1) + LO:512 * (t + 2)])

                # output slot t (row t+1): expectation = psum slot 9 cols of
                # every partition group. GPSIMD can't read PSUM, so this rides
                # DVE, issued after the latency-critical transposes.
                if t + 1 < PAST:
                    nc.vector.tensor_sub(outbuf[:, B * t:B * (t + 1)],
                                         ps_hi[:, 0:32],
                                         ywrap[:, B * t:B * (t + 1)])
                else:
                    nc.vector.tensor_copy(outbuf[:, B * t:B * (t + 1)],
                                          ps_hi[:, 0:32])
                if not last:
                    xT = nxT
                    ps_lo, ps_hi = nps_lo, nps_hi

                # stream finished output slices out while compute continues
                if (t + 1) % OUT_CHUNK == 0:
                    nc.sync.dma_start(
                        out=out_ext[:, B * out_done:B * (t + 1)],
                        in_=outbuf[:, B * out_done:B * (t + 1)])
                    out_done = t + 1

            if out_done < NSTEP:
                nc.sync.dma_start(out=out_ext[:, B * out_done:],
                                  in_=outbuf[:, B * out_done:])

    _order_preloads_after_output_reads(nc)
    _wire_preload_deps(nc)
    _thin_matmul_sems(nc)
    _split_multi_waits(nc)
    return nc


def _order_preloads_after_output_reads(nc):
    """The psum pool's scope tracking degrades for the rotated psum tiles
    (min-join warning), so the scheduler may move a hi-psum preload CAST
    ahead of the DVE sub/copy that still has to read the output slot of the
    buffer's previous occupant -- same physical bank, so that clobbers the
    output row. Both live on the DVE stream: delay each offending CAST to
    just after the reader it must follow (keyed by psum offset)."""
    for f in nc.m.functions:
        for b in f.blocks:
            insts = b.instructions
            unread = set()   # psum offsets cast-written but not yet sub-read
            pending = {}     # offset -> delayed cast instructions
            out = []
            changed = False
            for ins in insts:
                tn = type(ins).__name__
                delayed = False
                if tn in ("InstTensorCopy", "InstTensorTensor") \
                        and str(ins.engine).endswith("DVE"):
                    if tn == "InstTensorTensor":
                        o = ins.ins[0].offset
                        unread.discard(o)
                        out.append(ins)
                        out.extend(pending.pop(o, ()))
                        continue
                    oname = str(ins.outs[0].memref)
                    if oname.startswith("ob"):
                        o = ins.ins[0].offset
                        unread.discard(o)
                        out.append(ins)
                        out.extend(pending.pop(o, ()))
                        continue
                    if "ps_hi" in oname or "nps_hi" in oname:
                        o = ins.outs[0].offset
                        if o in unread:
                            pending.setdefault(o, []).append(ins)
                            delayed = True
                            changed = True
                        else:
                            unread.add(o)
                if not delayed:
                    out.append(ins)
            assert not pending, list(pending)
            if changed:
                b.instructions = out


def _strip_act_self_waits(nc):
    """Tile guards the th-tile WAR with a wait on the Activation engine's own
    counting semaphore, usually on the immediately preceding ACT. The engine
    executes serially anyway (exec queue depth 0), so the only effect is a
    ~30-130ns semaphore round-trip added to every tanh on the critical
    chain. Drop self-waits whose threshold is covered by program order."""
    act_sem = None
    blocks = [b for f in nc.m.functions for b in f.blocks]
    for b in blocks:
        for ins in b.instructions:
            if type(ins).__name__ == "InstActivation" and ins.sync_info:
                for u in ins.sync_info.on_update:
                    act_sem = u.id
                    break
            if act_sem is not None:
                break
        if act_sem is not None:
            break
    if act_sem is None:
        return
    from concourse import mybir
    issued = 0
    stripped = 0
    for b in blocks:
        for ins in b.instructions:
            if not str(ins.engine).endswith("Activation"):
                continue
            si = ins.sync_info
            if si and si.on_wait:
                kept = [w for w in si.on_wait
                        if not (w.id == act_sem and w.wait_value <= issued)]
                if len(kept) != len(si.on_wait):
                    stripped += len(si.on_wait) - len(kept)
                    ins.sync_info = mybir.SyncInfo(
                        on_wait=kept, on_update=list(si.on_update))
            if si:
                for u in si.on_update:
                    if u.id == act_sem:
                        issued += u.update_value


def _wire_preload_deps(nc):
    """Tile does not order a DVE PSUM preload (TensorCopy) against the
    matmuls that later accumulate onto the same buffer (it treats the matmul
    output as a pure write, so the cross-engine write->accumulate dependency
    is dropped and the preload can land mid-accumulation). Add the missing
    wait: the first matmul of each phase waits for the DVE counting-sem
    value reached by its preload copy."""
    import dataclasses

    blocks = [b for f in nc.m.functions for b in f.blocks]
    dve_sem = None
    for b in blocks:
        for ins in b.instructions:
            if type(ins).__name__ == "InstTensorCopy" and ins.sync_info \
                    and ins.sync_info.on_update:
                dve_sem = ins.sync_info.on_update[0].id
                break
        if dve_sem is not None:
            break
    assert dve_sem is not None

    template = None
    for b in blocks:
        for ins in b.instructions:
            si = ins.sync_info
            if si:
                for w in si.on_wait:
                    if w.id == dve_sem:
                        template = w
                        break
            if template:
                break
        if template:
            break
    assert template is not None

    act_sem = None
    for b in blocks:
        for ins in b.instructions:
            if type(ins).__name__ == "InstActivation" and ins.sync_info \
                    and ins.sync_info.on_update:
                act_sem = ins.sync_info.on_update[0].id
                break
        if act_sem is not None:
            break
    assert act_sem is not None
    act_template = None
    for b in blocks:
        for ins in b.instructions:
            si = ins.sync_info
            if si:
                for w in si.on_wait:
                    if w.id == act_sem:
                        act_template = w
        if act_template is not None:
            break
    assert act_template is not None

    from concourse import mybir

    def add_wait(ins, w):
        old = ins.sync_info or mybir.SyncInfo(on_wait=[], on_update=[])
        ins.sync_info = mybir.SyncInfo(
            on_wait=list(old.on_wait) + [w], on_update=list(old.on_update))

    dve_cnt = 0
    act_cnt = 0
    need_wait = {}
    # psum buffers alternate between instances 2 apart per tag; a preload
    # must also wait until the tanh reads of the instance two steps back are
    # done (write-after-read the scheduler can otherwise reorder across).
    hist = {"plo": [], "phi": []}
    act_read = {}
    wired = 0
    for b in blocks:
        for ins in b.instructions:
            si = ins.sync_info
            if si and str(ins.engine).endswith("DVE"):
                for u in si.on_update:
                    if u.id == dve_sem:
                        dve_cnt += u.update_value
            tn = type(ins).__name__
            if tn == "InstActivation":
                src = str(ins.ins[0].memref)
                if si:
                    for u in si.on_update:
                        if u.id == act_sem:
                            act_cnt += u.update_value
                act_read[src] = act_cnt
            elif tn == "InstTensorCopy":
                name = str(ins.outs[0].memref)
                if name.startswith(("ps_lo", "ps_hi", "nps_lo", "nps_hi")):
                    assert si and any(u.id == dve_sem for u in si.on_update), name
                    need_wait[name] = dve_cnt
                    key = "plo" if ("ps_lo" in name) else "phi"
                    hist[key].append(name)
                    if len(hist[key]) >= 3:
                        prev = hist[key][-3]
                        cnt = act_read.get(prev)
                        if cnt:
                            add_wait(ins, dataclasses.replace(
                                act_template, wait_value=cnt))
            elif tn == "InstMatmult":
                name = str(ins.outs[0].memref)
                if name in need_wait:
                    val = need_wait.pop(name)
                    add_wait(ins, dataclasses.replace(template, wait_value=val))
                    wired += 1
    assert wired >= 150, wired


def _thin_matmul_sems(nc):
    """Every matmul carries '++@complete' on the PE counting semaphore, and an
    instruction with semaphore ops costs ~34ns of PE sequencer time vs ~3ns
    without -- at 136 matmuls/step the sequencer, not the PE array, ends up
    pacing the kernel (~136ns/round floor). All waits on that semaphore sit
    exactly at stop-round boundaries, so only the 4 stop matmuls of each
    accumulation group need their updates: strip the rest and renumber every
    wait threshold from all-matmul counts to kept-update counts."""
    import dataclasses

    sem_ids = set()
    for f in nc.m.functions:
        for b in f.blocks:
            for ins in b.instructions:
                if type(ins).__name__ == "InstMatmult" and ins.sync_info:
                    for u in ins.sync_info.on_update:
                        sem_ids.add(u.id)
    if not sem_ids:
        return
    assert len(sem_ids) == 1, sem_ids
    sem = sem_ids.pop()

    mm_count = 0
    kept = 0
    remap = {}
    for f in nc.m.functions:
        for b in f.blocks:
            for ins in b.instructions:
                if type(ins).__name__ != "InstMatmult":
                    continue
                mm_count += 1
                si = ins.sync_info
                if ins.stop_tensor_calc:
                    kept += 1
                    remap[mm_count] = kept
                elif si is not None and si.on_update:
                    from concourse import mybir
                    ins.sync_info = mybir.SyncInfo(
                        on_wait=list(si.on_wait), on_update=[])

    for f in nc.m.functions:
        for b in f.blocks:
            for ins in b.instructions:
                si = ins.sync_info
                if si is None or not si.on_wait:
                    continue
                changed = False
                new_waits = []
                for w in si.on_wait:
                    if w.id == sem:
                        assert w.wait_value in remap, (
                            f"wait on PE sem at non-stop boundary: {w}")
                        new_waits.append(
                            dataclasses.replace(w, wait_value=remap[w.wait_value]))
                        changed = True
                    else:
                        new_waits.append(w)
                if changed:
                    from concourse import mybir
                    ins.sync_info = mybir.SyncInfo(
                        on_wait=new_waits, on_update=list(si.on_update))


def _split_multi_waits(nc):
    """This walrus build accepts at most one sem wait per instruction; Tile
    sometimes emits more. Hoist extras onto nops inserted just before the
    instruction in the same engine stream."""
    from concourse import mybir

    n = 0
    for f in nc.m.functions:
        for b in f.blocks:
            insts = b.instructions
            out = []
            changed = False
            for ins in insts:
                si = ins.sync_info
                if si is not None and len(si.on_wait) > 1:
                    waits = list(si.on_wait)
                    for w in waits[:-1]:
                        n += 1
                        out.append(mybir.InstNoOp(
                            name=f"I-waitsplit-{n}",
                            engine=ins.engine,
                            ins=[], outs=[],
                            bass_nofuse=True,
                            sync_info=mybir.SyncInfo(on_wait=[w], on_update=[]),
                        ))
                    ins.sync_info = mybir.SyncInfo(
                        on_wait=[waits[-1]], on_update=list(si.on_update))
                    changed = True
                out.append(ins)
            if changed:
                b.instructions = out


def _host_inputs(U, Y, A, init_state):
    """Build the per-core input maps (all pre-tanh / pre-transpose work)."""
    A = np.asarray(A, np.float32)
    U = np.asarray(U, np.float32)
    Y = np.asarray(Y, np.float32)
    init_state = np.asarray(init_state, np.float32)

    A_pad = np.zeros((KDIM, N_STATE), np.float16)
    A_pad[:N_STATE + N_U] = A.astype(np.float16)
    # column interleave: state col s (tile m=s//128, c=s%128) lands in
    # quadrant j=c//32 at free offset 32*slot(m) + c%32
    A_re = np.ascontiguousarray(
        A_pad.reshape(KDIM, 16, 4, 32).transpose(0, 2, 1, 3)[:, :, TILE_OF_SLOT, :]
        .reshape(KDIM, 4, 512))

    init_tanh = np.tanh(init_state[0]).astype(np.float16)          # (2048,)
    it = init_tanh.reshape(16, 128).T                              # (c, m)
    initxT = np.ascontiguousarray(
        np.broadcast_to(it[:, TILE_OF_SLOT][:, :, None],
                        (128, 16, 32)).reshape(128, 512))

    # exact y/u contributions to each step's matmul (fp32 host math): the
    # teacher-forced tile 0 (tanh y, past steps only) and the u rows, in
    # psum layout for the device's PSUM preload.
    Cu = (np.tanh(U[:NSTEP]).reshape(-1, N_U) @ A[N_STATE:]) \
        .reshape(NSTEP, BATCH, N_STATE)
    Cu[:PAST] += (np.tanh(Y).reshape(-1, N_Y) @ A[:N_Y]) \
        .reshape(PAST, BATCH, N_STATE)

    in_maps = []
    for c in range(N_CORES):
        b0 = c * B
        cb = (Cu[:, b0:b0 + B].reshape(NSTEP, B, 16, 4, 32)
              [:, :, TILE_OF_SLOT]                                 # (t, b, p, j, w)
              .transpose(3, 1, 0, 2, 4)                            # (j, b, t, p, w)
              .reshape(128, NSTEP * 512))
        # ywrap slot s (=1..63) at cols 32*(s-1): rows 32j+b = Y[s, b0+b, 32j+cc]
        yw = (Y[1:PAST, b0:b0 + B, :].reshape(PAST - 1, B, 4, 32)
              .transpose(0, 2, 1, 3)                               # (63, 4, 32b, 32cc)
              .reshape(PAST - 1, 128, 32)
              .transpose(1, 0, 2).reshape(128, (PAST - 1) * B))
        in_maps.append({
            "A_re": A_re,
            "contrib": np.ascontiguousarray(cb.astype(np.float16)),
            "ywrap": np.ascontiguousarray(yw.astype(np.float32)),
            "initxT": initxT,
        })
    return in_maps


def kernel(U, Y, A, init_state):
    from concourse.bass_utils import run_bass_kernel_spmd

    nc = _build_program()
    in_maps = _host_inputs(U, Y, A, init_state)
    res = run_bass_kernel_spmd(nc, in_maps, list(range(N_CORES)))

    out = np.empty((T, BATCH, N_Y), np.float32)
    # slot 0: err for t=0 is pure host math (state_0 = broadcast init_state)
    out[0] = np.asarray(init_state, np.float32)[0, :N_Y][None, :] - np.asarray(Y, np.float32)[0]
    for c in range(N_CORES):
        b0 = c * B
        ob = res.results[c]["outbuf"]                              # (128, 79*32)
        # [32j+b, 32t+cc] = out[t+1, b0+b, 32j+cc]
        ob4 = ob.reshape(4, 32, NSTEP, 32)                         # (j, b, t, cc)
        out[1:, b0:b0 + B, :] = ob4.transpose(2, 1, 0, 3).reshape(NSTEP, B, N_Y)
    return out


if __name__ == "__main__":
    rng = np.random.default_rng(0)
    U = rng.standard_normal((T, BATCH, N_U)).astype(np.float32)
    Y = rng.standard_normal((PAST, BATCH, N_Y)).astype(np.float32)
    A = (rng.standard_normal((N_STATE + N_U, N_STATE)) * 0.02).astype(np.float32)
    init = rng.standard_normal((1, N_STATE)).astype(np.float32)
    o = kernel(U=U, Y=Y, A=A, init_state=init)
    print("kernel out:", o.shape, o.dtype)


# revision 40
# speedup vs baseline: 1.0329x; 1.0329x over previous
"""HCNN (known-U) recurrence kernel for 8 Trainium2 NeuronCores.

Model (see reference): 80 sequential steps of
    state' = tanh(cat(post_state, u)) @ A            A: (2112, 2048) fp32
with teacher forcing post_state[:, :128] = y during the 64 past steps,
outputs = 64 past errors then 16 forecasts (first 128 state components).

Strategy
--------
Data-parallel over batch: 256 = 8 cores x 32. Each core runs the full
recurrence for its batch slice; no collectives.

Per-core per-step matmul x @ A with batch M=32 would waste 3/4 of the
128-wide PE array, so the A columns are split into 4 interleaved groups
and computed by 4 concurrent column-tiled matmuls (tile_position=(0,32j))
sharing the array. Data is fp16 (single pass): the teacher-forced
recurrence is strongly contractive; end-to-end output error ~1.5e-4
relative vs the fp32 reference.

Column interleave: state column s lives in col-group j=(s//32)%4 at free
offset 32*(s//128) + s%32. The (128, 512) psum holding state' (batch on
partitions within each 32-group) turns into the next step's stationary
operand layout via DVE 32x32 block-transposes: block (j, m') lands at
partitions [32j:32j+32] of k-tile m' -- exactly where matmul round m'
reads it.

Pipelining: each step's state k-rounds are split into lo (psum cols
0:288, slot-permuted to hold tiles 1..9 of the next x) and hi (cols
288:512, tile 0 + tiles 10..15) accumulation groups. The lo group stops
half way through the step, so ACT tanh + DVE transpose of the lo chunks
overlap the hi matmuls, and the hi chunks' tanh/transpose overlaps the
next step's early rounds (which consume lo-chunk operands first). The
y/u contributions to every step are computed exactly on the host and
DVE-preloaded into PSUM (start=False accumulation), removing their
matmul rounds entirely. Post-passes patch the Tile-generated sync: the
missing cross-engine preload->accumulate dependency is wired manually,
and the per-matmul semaphore updates are thinned to the stop rounds
(an instruction with semaphore ops costs ~34ns of PE sequencer time vs
~3ns without, which would otherwise cap the round rate below the PE
array's streaming rate). A tiles are DMA-streamed in first-use order so
step 0 starts as soon as its first tiles land; outputs are DMA'd out in
slices as they are produced.
"""

import sys

for _p in ("/opt/trn_rl_repo", "/root/.axon_site/_ro/trn_rl_repo"):
    if _p not in sys.path:
        sys.path.insert(0, _p)

import numpy as np

N_STATE = 2048
N_U = 64
N_Y = 128
PAST = 64
FORE = 16
BATCH = 256
T = PAST + FORE          # 80 total steps; only 79 matmul steps needed
NSTEP = T - 1            # step t computes state_{t+1}; state_80 is unused
NK = 17                  # contraction tiles: 16 x 128 state + 1 x (64 u + 64 pad)
KDIM = NK * 128          # 2176 padded contraction size
N_CORES = 8
B = BATCH // N_CORES     # 32 per core
OUT_CHUNK = 10           # output DMA granularity in steps

# psum slot permutation: slot p of the 16 32-col psum slots holds state
# k-tile TILE_OF_SLOT[p]. Tiles 1..9 form the lo accumulation group (cols
# 0:288); tile 0 (the output/forecast tile) leads the hi group, 10..15 trail.
TILE_OF_SLOT = [1, 2, 3, 4, 5, 6, 7, 8, 9, 0, 10, 11, 12, 13, 14, 15]
SLOT_OF_TILE = [0] * 16
for _p, _m in enumerate(TILE_OF_SLOT):
    SLOT_OF_TILE[_m] = _p
LO = 288                 # lo accumulation group column count


def _build_program():
    import concourse.bass as bass
    import concourse.tile as tile
    from concourse import mybir

    F32 = mybir.dt.float32
    F16 = mybir.dt.float16

    nc = bass.Bass("TRN2", target_bir_lowering=False, debug=False,
                   num_devices=N_CORES)

    A_ext = nc.declare_dram_parameter("A_re", [KDIM, 4, 512], F16, isOutput=False)
    contrib_ext = nc.declare_dram_parameter("contrib", [128, NSTEP * 512], F16, isOutput=False)
    ywrap_ext = nc.declare_dram_parameter("ywrap", [128, (PAST - 1) * B], F32, isOutput=False)
    initxT_ext = nc.declare_dram_parameter("initxT", [128, 512], F16, isOutput=False)
    out_ext = nc.declare_dram_parameter("outbuf", [128, NSTEP * B], F32, isOutput=True)

    # The y/u contributions to every step are known ahead of time; the host
    # computes them exactly and they are DVE-preloaded into PSUM, so the
    # device rounds cover only the state tiles (start=False accumulation on
    # top of the preload). The lo group's tiles are produced and
    # tanh/transposed a half-phase early; the k emission order consumes them
    # first, giving the late hi chunks ~1.2us of cover for the
    # stop->tanh->transpose chain.
    korder_past = list(range(1, 16))
    korder_fore = list(range(1, 10)) + [0] + list(range(10, 16))
    CCH = 8 * 512            # contrib DMA chunk: 8 steps

    with tile.TileContext(nc) as tc:
        with tc.tile_pool(name="const", bufs=1) as cpool, \
             tc.tile_pool(name="xbuf", bufs=2) as xpool, \
             tc.tile_pool(name="th", bufs=2) as thpool, \
             tc.tile_pool(name="psum", bufs=2, space="PSUM") as pspool:

            xT = xpool.tile([128, 512], F16, tag="xT")
            nc.sync.dma_start(out=xT[:], in_=initxT_ext[:])
            contrib = cpool.tile([128, NSTEP * 512], F16, tag="cb")
            # step 0's slice alone first: the initial PSUM preloads (and with
            # them all of step 0) would otherwise wait on a megabyte chunk
            nc.sync.dma_start(out=contrib[:, 0:512], in_=contrib_ext[:, 0:512])
            nc.sync.dma_start(out=contrib[:, 512:CCH], in_=contrib_ext[:, 512:CCH])

            # A k-tiles as separate tiles, DMA'd in first-use order so each
            # round of step 0 waits only for its own tile. Tile 0 is only
            # used from the first forecast-input step, so it loads last.
            A_sb = [None] * NK
            for i, k in enumerate(korder_past + [0]):
                A_sb[k] = cpool.tile([128, 2048], F16, tag=f"A{k}", name=f"A{k}")
                nc.sync.dma_start(out=A_sb[k][:],
                                  in_=A_ext[128 * k:128 * (k + 1), :, :])
                if i == 8:
                    nc.sync.dma_start(out=contrib[:, CCH:2 * CCH],
                                      in_=contrib_ext[:, CCH:2 * CCH])

            ywrap = cpool.tile([128, (PAST - 1) * B], F32, tag="yw")
            nc.sync.dma_start(out=ywrap[:], in_=ywrap_ext[:])
            for c in range(2, (NSTEP * 512 + CCH - 1) // CCH):
                nc.sync.dma_start(
                    out=contrib[:, c * CCH:min((c + 1) * CCH, NSTEP * 512)],
                    in_=contrib_ext[:, c * CCH:min((c + 1) * CCH, NSTEP * 512)])
            outbuf = cpool.tile([128, NSTEP * B], F32, tag="ob")

            def lhs_for(k, x):
                p = SLOT_OF_TILE[k]
                return x[:, 32 * p:32 * (p + 1)]

            ps_lo = pspool.tile([128, 512], F32, tag="plo")
            ps_hi = pspool.tile([128, 512], F32, tag="phi")
            nc.vector.tensor_copy(ps_lo[:, 0:LO], contrib[:, 0:LO])
            nc.vector.tensor_copy(ps_hi[:, 0:512 - LO], contrib[:, LO:512])

            out_done = 0
            for t in range(NSTEP):
                korder = korder_past if t < PAST else korder_fore
                last = t == NSTEP - 1
                if not last:
                    nxT = xpool.tile([128, 512], F16, tag="xT", name="nxT")
                    nps_lo = pspool.tile([128, 512], F32, tag="plo", name="nps_lo")
                    nps_hi = pspool.tile([128, 512], F32, tag="phi", name="nps_hi")

                # the last step only needs output cols 0:128 (= psum slot 9 =
                # hi cols 0:32 of each quadrant); its lo half is never run.
                halves = ((0, ps_lo), (1, ps_hi)) if not last else ((1, ps_hi),)
                for half, ps in halves:
                    c0, w = (0, LO) if half == 0 else (LO, 512 - LO)
                    for idx, k in enumerate(korder):
                        lhsT = lhs_for(k, xT)
                        stop = idx == len(korder) - 1
                        for j in range(4):
                            nc.tensor.matmul(
                                ps[32 * j:32 * (j + 1), 0:w],
                                lhsT,
                                A_sb[k][:, 512 * j + c0:512 * j + c0 + w],
                                start=False, stop=stop,
                                tile_position=(0, 32 * j),
                            )
                    if last:
                        continue  # last state: only the output copy is needed
                    if half == 0 and t + 1 < NSTEP - 1:
                        # preload next step's lo psum while this step's hi
                        # phase runs (DVE, ahead of the transposes: those of
                        # the lo half have a full phase of slack)
                        nc.vector.tensor_copy(
                            nps_lo[:, 0:LO],
                            contrib[:, 512 * (t + 1):512 * (t + 1) + LO])
                    # tanh + 32x32 block transpose of this half's slots into
                    # the next step's stationary operand tile. The hi chunks
                    # are on the next step's critical path: keep them 64 cols
                    # for low latency. Slot 9 (tile 0) is only needed once
                    # teacher forcing ends (t >= PAST-1).
                    if half == 0:
                        chunks = ((0, 128), (128, 288))
                    elif t < PAST - 1:
                        chunks = ((320, 384), (384, 448), (448, 512))
                    else:
                        chunks = ((288, 352), (352, 416), (416, 480), (480, 512))
                    for a, b_ in chunks:
                        th = thpool.tile([128, b_ - a], F16, tag=f"th{half}{a}")
                        nc.scalar.activation(th[:], ps[:, a - c0:b_ - c0],
                                             mybir.ActivationFunctionType.Tanh)
                        nc.vector.transpose(nxT[:, a:b_], th[:])
                if not last:
                    nc.vector.tensor_copy(
                        nps_hi[:, 0:512 - LO],
                        contrib[:, 512 * (t + 1) + LO:512 * (t + 2)])

                # output slot t (row t+1): expectation = psum slot 9 cols of
                # every partition group. GPSIMD can't read PSUM, so this rides
                # DVE, issued after the latency-critical transposes.
                if t + 1 < PAST:
                    nc.vector.tensor_sub(outbuf[:, B * t:B * (t + 1)],
                                         ps_hi[:, 0:32],
                                         ywrap[:, B * t:B * (t + 1)])
                else:
                    nc.vector.tensor_copy(outbuf[:, B * t:B * (t + 1)],
                                          ps_hi[:, 0:32])
                if not last:
                    xT = nxT
                    ps_lo, ps_hi = nps_lo, nps_hi

                # stream finished output slices out while compute continues
                if (t + 1) % OUT_CHUNK == 0:
                    nc.sync.dma_start(
                        out=out_ext[:, B * out_done:B * (t + 1)],
                        in_=outbuf[:, B * out_done:B * (t + 1)])
                    out_done = t + 1

            if out_done < NSTEP:
                nc.sync.dma_start(out=out_ext[:, B * out_done:],
                                  in_=outbuf[:, B * out_done:])

    _wire_preload_deps(nc)
    _thin_matmul_sems(nc)
    _split_multi_waits(nc)
    return nc


def _order_preloads_after_output_reads(nc):
    """The psum pool's scope tracking degrades for the rotated psum tiles
    (min-join warning), so the scheduler may move a hi-psum preload CAST
    ahead of the DVE sub/copy that still has to read the output slot of the
    buffer's previous occupant -- same physical bank, so that clobbers the
    output row. Both live on the DVE stream: delay each offending CAST to
    just after the reader it must follow (keyed by psum offset)."""
    # the pool rotates 2 buffers per tag in tile-creation order; creation
    # order is recoverable from the monotonically increasing name suffix
    def suffix(name):
        return int(name.rsplit("_", 1)[1])

    phi_names = set()
    for f in nc.m.functions:
        for b in f.blocks:
            for ins in b.instructions:
                if type(ins).__name__ == "InstTensorCopy":
                    o = str(ins.outs[0].memref)
                    if "ps_hi" in o or "nps_hi" in o:
                        phi_names.add(o)
    buf_of = {n: i % 2 for i, n in enumerate(sorted(phi_names, key=suffix))}

    for f in nc.m.functions:
        for b in f.blocks:
            insts = b.instructions
            unread = set()   # buffers cast-written but not yet sub-read
            pending = {}     # buffer -> delayed cast instructions
            out = []
            changed = False
            for ins in insts:
                tn = type(ins).__name__
                delayed = False
                if tn in ("InstTensorCopy", "InstTensorTensor") \
                        and str(ins.engine).endswith("DVE"):
                    rd = None
                    if tn == "InstTensorTensor":
                        rd = str(ins.ins[0].memref)
                    else:
                        oname = str(ins.outs[0].memref)
                        if oname.startswith("ob"):
                            rd = str(ins.ins[0].memref)
                        elif oname in buf_of:
                            o = buf_of[oname]
                            if o in unread:
                                pending.setdefault(o, []).append(ins)
                                delayed = True
                                changed = True
                            else:
                                unread.add(o)
                    if rd is not None and rd in buf_of:
                        o = buf_of[rd]
                        unread.discard(o)
                        out.append(ins)
                        rel = pending.pop(o, ())
                        assert len(rel) <= 1, rel
                        if rel:
                            out.extend(rel)
                            unread.add(o)
                        continue
                if not delayed:
                    out.append(ins)
            assert not pending, list(pending)
            if changed:
                b.instructions = out


def _strip_act_self_waits(nc):
    """Tile guards the th-tile WAR with a wait on the Activation engine's own
    counting semaphore, usually on the immediately preceding ACT. The engine
    executes serially anyway (exec queue depth 0), so the only effect is a
    ~30-130ns semaphore round-trip added to every tanh on the critical
    chain. Drop self-waits whose threshold is covered by program order."""
    act_sem = None
    blocks = [b for f in nc.m.functions for b in f.blocks]
    for b in blocks:
        for ins in b.instructions:
            if type(ins).__name__ == "InstActivation" and ins.sync_info:
                for u in ins.sync_info.on_update:
                    act_sem = u.id
                    break
            if act_sem is not None:
                break
        if act_sem is not None:
            break
    if act_sem is None:
        return
    from concourse import mybir
    issued = 0
    stripped = 0
    for b in blocks:
        for ins in b.instructions:
            if not str(ins.engine).endswith("Activation"):
                continue
            si = ins.sync_info
            if si and si.on_wait:
                kept = [w for w in si.on_wait
                        if not (w.id == act_sem and w.wait_value <= issued)]
                if len(kept) != len(si.on_wait):
                    stripped += len(si.on_wait) - len(kept)
                    ins.sync_info = mybir.SyncInfo(
                        on_wait=kept, on_update=list(si.on_update))
            if si:
                for u in si.on_update:
                    if u.id == act_sem:
                        issued += u.update_value


def _wire_preload_deps(nc):
    """Tile does not order a DVE PSUM preload (TensorCopy) against the
    matmuls that later accumulate onto the same buffer (it treats the matmul
    output as a pure write, so the cross-engine write->accumulate dependency
    is dropped and the preload can land mid-accumulation). Add the missing
    wait: the first matmul of each phase waits for the DVE counting-sem
    value reached by its preload copy."""
    import dataclasses

    blocks = [b for f in nc.m.functions for b in f.blocks]
    dve_sem = None
    for b in blocks:
        for ins in b.instructions:
            if type(ins).__name__ == "InstTensorCopy" and ins.sync_info \
                    and ins.sync_info.on_update:
                dve_sem = ins.sync_info.on_update[0].id
                break
        if dve_sem is not None:
            break
    assert dve_sem is not None

    template = None
    for b in blocks:
        for ins in b.instructions:
            si = ins.sync_info
            if si:
                for w in si.on_wait:
                    if w.id == dve_sem:
                        template = w
                        break
            if template:
                break
        if template:
            break
    assert template is not None

    from concourse import mybir
    dve_cnt = 0
    need_wait = {}
    wired = 0
    for b in blocks:
        for ins in b.instructions:
            si = ins.sync_info
            if si and str(ins.engine).endswith("DVE"):
                for u in si.on_update:
                    if u.id == dve_sem:
                        dve_cnt += u.update_value
            tn = type(ins).__name__
            if tn == "InstTensorCopy":
                name = str(ins.outs[0].memref)
                if name.startswith(("ps_lo", "ps_hi", "nps_lo", "nps_hi")):
                    assert si and any(u.id == dve_sem for u in si.on_update), name
                    need_wait[name] = dve_cnt
            elif tn == "InstMatmult":
                name = str(ins.outs[0].memref)
                if name in need_wait:
                    val = need_wait.pop(name)
                    w = dataclasses.replace(template, wait_value=val)
                    old = ins.sync_info or mybir.SyncInfo(on_wait=[], on_update=[])
                    ins.sync_info = mybir.SyncInfo(
                        on_wait=list(old.on_wait) + [w],
                        on_update=list(old.on_update))
                    wired += 1
    assert wired >= 150, wired


def _thin_matmul_sems(nc):
    """Every matmul carries '++@complete' on the PE counting semaphore, and an
    instruction with semaphore ops costs ~34ns of PE sequencer time vs ~3ns
    without -- at 136 matmuls/step the sequencer, not the PE array, ends up
    pacing the kernel (~136ns/round floor). All waits on that semaphore sit
    exactly at stop-round boundaries, so only the 4 stop matmuls of each
    accumulation group need their updates: strip the rest and renumber every
    wait threshold from all-matmul counts to kept-update counts."""
    import dataclasses

    sem_ids = set()
    for f in nc.m.functions:
        for b in f.blocks:
            for ins in b.instructions:
                if type(ins).__name__ == "InstMatmult" and ins.sync_info:
                    for u in ins.sync_info.on_update:
                        sem_ids.add(u.id)
    if not sem_ids:
        return
    assert len(sem_ids) == 1, sem_ids
    sem = sem_ids.pop()

    mm_count = 0
    kept = 0
    remap = {}
    for f in nc.m.functions:
        for b in f.blocks:
            for ins in b.instructions:
                if type(ins).__name__ != "InstMatmult":
                    continue
                mm_count += 1
                si = ins.sync_info
                if ins.stop_tensor_calc:
                    kept += 1
                    remap[mm_count] = kept
                elif si is not None and si.on_update:
                    from concourse import mybir
                    ins.sync_info = mybir.SyncInfo(
                        on_wait=list(si.on_wait), on_update=[])

    for f in nc.m.functions:
        for b in f.blocks:
            for ins in b.instructions:
                si = ins.sync_info
                if si is None or not si.on_wait:
                    continue
                changed = False
                new_waits = []
                for w in si.on_wait:
                    if w.id == sem:
                        assert w.wait_value in remap, (
                            f"wait on PE sem at non-stop boundary: {w}")
                        new_waits.append(
                            dataclasses.replace(w, wait_value=remap[w.wait_value]))
                        changed = True
                    else:
                        new_waits.append(w)
                if changed:
                    from concourse import mybir
                    ins.sync_info = mybir.SyncInfo(
                        on_wait=new_waits, on_update=list(si.on_update))


def _split_multi_waits(nc):
    """This walrus build accepts at most one sem wait per instruction; Tile
    sometimes emits more. Hoist extras onto nops inserted just before the
    instruction in the same engine stream."""
    from concourse import mybir

    n = 0
    for f in nc.m.functions:
        for b in f.blocks:
            insts = b.instructions
            out = []
            changed = False
            for ins in insts:
                si = ins.sync_info
                if si is not None and len(si.on_wait) > 1:
                    waits = list(si.on_wait)
                    for w in waits[:-1]:
                        n += 1
                        out.append(mybir.InstNoOp(
                            name=f"I-waitsplit-{n}",
                            engine=ins.engine,
                            ins=[], outs=[],
                            bass_nofuse=True,
                            sync_info=mybir.SyncInfo(on_wait=[w], on_update=[]),
                        ))
                    ins.sync_info = mybir.SyncInfo(
                        on_wait=[waits[-1]], on_update=list(si.on_update))
                    changed = True
                out.append(ins)
            if changed:
                b.instructions = out


def _host_inputs(U, Y, A, init_state):
    """Build the per-core input maps (all pre-tanh / pre-transpose work)."""
    A = np.asarray(A, np.float32)
    U = np.asarray(U, np.float32)
    Y = np.asarray(Y, np.float32)
    init_state = np.asarray(init_state, np.float32)

    A_pad = np.zeros((KDIM, N_STATE), np.float16)
    A_pad[:N_STATE + N_U] = A.astype(np.float16)
    # column interleave: state col s (tile m=s//128, c=s%128) lands in
    # quadrant j=c//32 at free offset 32*slot(m) + c%32
    A_re = np.ascontiguousarray(
        A_pad.reshape(KDIM, 16, 4, 32).transpose(0, 2, 1, 3)[:, :, TILE_OF_SLOT, :]
        .reshape(KDIM, 4, 512))

    init_tanh = np.tanh(init_state[0]).astype(np.float16)          # (2048,)
    it = init_tanh.reshape(16, 128).T                              # (c, m)
    initxT = np.ascontiguousarray(
        np.broadcast_to(it[:, TILE_OF_SLOT][:, :, None],
                        (128, 16, 32)).reshape(128, 512))

    # exact y/u contributions to each step's matmul (fp32 host math): the
    # teacher-forced tile 0 (tanh y, past steps only) and the u rows, in
    # psum layout for the device's PSUM preload.
    Cu = (np.tanh(U[:NSTEP]).reshape(-1, N_U) @ A[N_STATE:]) \
        .reshape(NSTEP, BATCH, N_STATE)
    Cu[:PAST] += (np.tanh(Y).reshape(-1, N_Y) @ A[:N_Y]) \
        .reshape(PAST, BATCH, N_STATE)

    in_maps = []
    for c in range(N_CORES):
        b0 = c * B
        cb = (Cu[:, b0:b0 + B].reshape(NSTEP, B, 16, 4, 32)
              [:, :, TILE_OF_SLOT]                                 # (t, b, p, j, w)
              .transpose(3, 1, 0, 2, 4)                            # (j, b, t, p, w)
              .reshape(128, NSTEP * 512))
        # ywrap slot s (=1..63) at cols 32*(s-1): rows 32j+b = Y[s, b0+b, 32j+cc]
        yw = (Y[1:PAST, b0:b0 + B, :].reshape(PAST - 1, B, 4, 32)
              .transpose(0, 2, 1, 3)                               # (63, 4, 32b, 32cc)
              .reshape(PAST - 1, 128, 32)
              .transpose(1, 0, 2).reshape(128, (PAST - 1) * B))
        in_maps.append({
            "A_re": A_re,
            "contrib": np.ascontiguousarray(cb.astype(np.float16)),
            "ywrap": np.ascontiguousarray(yw.astype(np.float32)),
            "initxT": initxT,
        })
    return in_maps


def kernel(U, Y, A, init_state):
    from concourse.bass_utils import run_bass_kernel_spmd

    nc = _build_program()
    in_maps = _host_inputs(U, Y, A, init_state)
    res = run_bass_kernel_spmd(nc, in_maps, list(range(N_CORES)))

    out = np.empty((T, BATCH, N_Y), np.float32)
    # slot 0: err for t=0 is pure host math (state_0 = broadcast init_state)
    out[0] = np.asarray(init_state, np.float32)[0, :N_Y][None, :] - np.asarray(Y, np.float32)[0]
    for c in range(N_CORES):
        b0 = c * B
        ob = res.results[c]["outbuf"]                              # (128, 79*32)
        # [32j+b, 32t+cc] = out[t+1, b0+b, 32j+cc]
        ob4 = ob.reshape(4, 32, NSTEP, 32)                         # (j, b, t, cc)
        out[1:, b0:b0 + B, :] = ob4.transpose(2, 1, 0, 3).reshape(NSTEP, B, N_Y)
    return out


if __name__ == "__main__":
    rng = np.random.default_rng(0)
    U = rng.standard_normal((T, BATCH, N_U)).astype(np.float32)
    Y = rng.standard_normal((PAST, BATCH, N_Y)).astype(np.float32)
    A = (rng.standard_normal((N_STATE + N_U, N_STATE)) * 0.02).astype(np.float32)
    init = rng.standard_normal((1, N_STATE)).astype(np.float32)
    o = kernel(U=U, Y=Y, A=A, init_state=init)
    print("kernel out:", o.shape, o.dtype)
